# revision 13
# baseline (speedup 1.0000x reference)
"""Trainium2 Bass kernel for MultiHeadGlobalAttention2d.

Sharding (8 cores): core = (batch b, head-group g), b in 0..3, g in 0..1.
Each core computes, for its batch and its 4 heads (128 channels):
  q/k/v projections, attention (softmax over keys), and the partial output
  projection  y_part = Wo[:, ch_slice] @ att_out.
Host sums the two partials per batch and adds the output bias.

v3 design (ACT-exp is the bottleneck engine; keep it 100% busy):
  - S^T orientation: score tiles [keys(m) on partitions, queries(n) on free];
    exp on ScalarE straight out of PSUM; denominators via ones-stationary
    matmuls accumulated alongside AV.
  - All projections in f16 (full PE rate + FWL); x inputs DMA'd as f16 in
    512-column chunks spread over many DMA queues so projections and the
    attention stream start within ~10us.
  - vT produced directly by PE matmul (x^T stationary, Wv^T moving) - no
    PE transposes. Biases added by DVE during the PSUM->SBUF copies.
  - Per-head accumulator open/close (start on j==0, stop on j==NMT-1).
  - PSUM: 3 score slots [128,1024] (6 banks) + 2 accumulator banks (AV+den).
    Leftover projection/vT work inside block 0 borrows score slots in
    batched groups (4 vT chunks or 2 proj blocks per borrow) to keep the
    QK lookahead intact.
  - Block o's epilogue (recip_approx + mul + out-proj) is emitted inside
    block o+1 so the exp stream never stalls at block boundaries.
"""

import numpy as np

B = 4
CIN = 256
COUT = 256
HH = 48
WW = 48
N = HH * WW            # 2304
D = 32                 # head dim
NHL = 4                # heads per core
HGC = NHL * D          # 128 channels per head-group
NCORES = 8
NBLK = [(0, 512), (512, 512), (1024, 512), (1536, 512), (2048, 256)]
NMT = N // 128         # 18 key tiles
XCH = [(0, 512), (512, 512), (1024, 512), (1536, 512), (2048, 256)]  # DMA chunks

_PROG = {}


def build_program():
    if "nc" in _PROG:
        return _PROG["nc"]

    from contextlib import ExitStack

    import concourse.bacc as bacc
    import concourse.mybir as mybir
    import concourse.tile as tile

    f32 = mybir.dt.float32
    bf16 = mybir.dt.bfloat16
    f16 = mybir.dt.float16
    i16 = mybir.dt.int16
    EXP = mybir.ActivationFunctionType.Exp
    MULT = mybir.AluOpType.mult
    ADD = mybir.AluOpType.add
    # DVE fast-exp: bitcast(int16(s*(128*log2e)/16 + (127*128 - C))) read as
    # bf16 approximates exp(s/16) (Schraudolph); C tuned for min max-rel-err.
    FE_A = 128.0 * 1.4426950408889634 / 16.0
    FE_B = 127.0 * 128.0 - 5.5

    nc = bacc.Bacc("TRN2", target_bir_lowering=False, debug=False)

    xq_d = nc.declare_dram_parameter("xq", [CIN, N], f16, False)
    xk_d = nc.declare_dram_parameter("xk", [CIN, N], f16, False)
    xv_d = nc.declare_dram_parameter("xv", [CIN, N], f16, False)
    wqT_d = nc.declare_dram_parameter("wqT", [CIN, HGC], f16, False)
    wkT_d = nc.declare_dram_parameter("wkT", [CIN, HGC], f16, False)
    wvT_d = nc.declare_dram_parameter("wvT", [CIN, HGC], f16, False)
    woA_d = nc.declare_dram_parameter("woA", [128, COUT], bf16, False)
    woB_d = nc.declare_dram_parameter("woB", [128, COUT], bf16, False)
    bq_d = nc.declare_dram_parameter("bq", [HGC, 1], f32, False)
    bk_d = nc.declare_dram_parameter("bk", [HGC, 1], f32, False)
    bvrep_d = nc.declare_dram_parameter("bvrep", [128, HGC], f16, False)
    y_d = nc.declare_dram_parameter("y", [COUT, N], f32, True)

    with tile.TileContext(nc) as tc, ExitStack() as ctx:
        const = ctx.enter_context(tc.tile_pool(name="const", bufs=1))
        resid = ctx.enter_context(tc.tile_pool(name="resid", bufs=1))
        xin = ctx.enter_context(tc.tile_pool(name="xin", bufs=1))
        espool = ctx.enter_context(tc.tile_pool(name="espool", bufs=12))
        trans = ctx.enter_context(tc.tile_pool(name="trans", bufs=2))
        ps_s = ctx.enter_context(tc.tile_pool(name="ps_s", bufs=3, space="PSUM"))
        ps_a = ctx.enter_context(tc.tile_pool(name="ps_a", bufs=2, space="PSUM"))

        # ---- constants / weights (small, issued first) ----
        wq = const.tile([128, 2, 128], f16)
        wk = const.tile([128, 2, 128], f16)
        wv = const.tile([128, 2, 128], f16)
        for wt, wd in ((wk, wkT_d), (wq, wqT_d), (wv, wvT_d)):
            nc.sync.dma_start(wt[:, :, :], wd[:, :].rearrange("(c p) k -> p c k", p=128))
        woA = const.tile([128, COUT], bf16)
        woB = const.tile([128, COUT], bf16)
        nc.sync.dma_start(woA[:, :], woA_d[:, :])
        nc.sync.dma_start(woB[:, :], woB_d[:, :])
        bq_s = const.tile([128, 1], f32)
        bk_s = const.tile([128, 1], f32)
        for bt, bd in ((bk_s, bk_d), (bq_s, bq_d)):
            nc.sync.dma_start(bt[:, :], bd[:, :])
        bvrep = const.tile([128, 128], f16)
        nc.sync.dma_start(bvrep[:, :], bvrep_d[:, :])

        # ---- x inputs: 512-col chunks, k/q/v interleaved so the front of
        # each tensor lands early and chunks spread across DMA queues ----
        xk_t = xin.tile([128, 2, N], f16)
        xq_t = xin.tile([128, 2, N], f16)
        xv_t = xin.tile([128, 2, N], f16)
        for lo, sz in XCH:
            for xt, xd in ((xk_t, xk_d), (xq_t, xq_d), (xv_t, xv_d)):
                nc.sync.dma_start(
                    xt[:, :, lo : lo + sz],
                    xd[:, lo : lo + sz].rearrange("(c p) n -> p c n", p=128),
                )

        # ---- residents ----
        q_sb = resid.tile([128, N], f16)
        k_sb = resid.tile([128, N], f16)
        vT2_sb = resid.tile([128, 2 * N], bf16)
        recTA = resid.tile([128, 512], f32)
        recTB = resid.tile([128, 512], f32)
        attA_sb = resid.tile([128, 512], bf16)
        attB_sb = resid.tile([128, 512], bf16)
        # ones strips (cols 32:64 of each 64-col head group), written once;
        # att junk lanes (den positions) stay zero forever
        nc.vector.memset(
            vT2_sb[:, :].rearrange("p (g t) -> p g t", t=64)[:, :, D:64], 1.0)
        nc.vector.memset(attA_sb[:, :], 0.0)
        nc.vector.memset(attB_sb[:, :], 0.0)

        # PE prologue absorb for const tiles consumed by PE (x chunks carry
        # their DMA wait on the consuming matmul directly).
        def absorb(tiles):
            scr = ps_s.tile([128, 512], f32, tag="s", name="scr")
            for t in tiles:
                tv = t[:, :, :].rearrange("p c k -> p (c k)") if t.ndim == 3 else t[:, :]
                F = tv.shape[-1]
                M = min(F, 128)
                W = min(F, 2)
                nc.tensor.matmul(
                    scr[0:M, 0:W], tv[0:1, F - M : F], tv[0:1, F - W : F],
                    start=True, stop=True,
                )

        def proj_piece(pp, po_, w, xt, brow, dst, o, sz):
            """dst[:, o:o+sz] = (W_h @ x)[:, o:o+sz] + b ; pp[:, po_:po_+sz] scratch."""
            nc.tensor.matmul(
                pp[:, po_ : po_ + sz], w[:, 0, :], xt[:, 0, o : o + sz],
                start=True, stop=False,
            )
            nc.tensor.matmul(
                pp[:, po_ : po_ + sz], w[:, 1, :], xt[:, 1, o : o + sz],
                start=False, stop=True,
            )
            nc.vector.tensor_scalar_add(dst[:, o : o + sz], pp[:, po_ : po_ + sz], brow[:, 0:1])

        def vt_piece(pt, po_, j):
            """vT_sb[:, 128j:+128] = (x_v[:, 128j:+128])^T @ Wv^T + bv."""
            nc.tensor.matmul(
                pt[:, po_ : po_ + 128], xv_t[:, 0, 128 * j : 128 * j + 128], wv[:, 0, :],
                start=True, stop=False,
            )
            nc.tensor.matmul(
                pt[:, po_ : po_ + 128], xv_t[:, 1, 128 * j : 128 * j + 128], wv[:, 1, :],
                start=False, stop=True,
            )
            vstrip = vT2_sb[:, 256 * j : 256 * j + 256].rearrange(
                "p (h t) -> p h t", h=NHL)[:, :, 0:D]
            nc.vector.tensor_add(
                vstrip,
                pt[:, po_ : po_ + 128].rearrange("p (h t) -> p h t", h=NHL),
                bvrep[:, :].rearrange("p (h t) -> p h t", h=NHL),
            )

        def vt_group(js):
            pt = ps_s.tile([128, 1024], f32, tag="s", name="pt")
            for i, j in enumerate(js):
                vt_piece(pt, 128 * i, j)

        def proj_group(pieces):
            pp = ps_s.tile([128, 1024], f32, tag="s", name="pp")
            po_ = 0
            for w, xt, brow, dst, o, sz in pieces:
                proj_piece(pp, po_, w, xt, brow, dst, o, sz)
                po_ += sz

        # ---- prologue: minimum to start the exp stream ----
        absorb([wk, wq, bk_s, bq_s])
        proj_group([(wk, xk_t, bk_s, k_sb, 0, 512)])
        proj_group([(wq, xq_t, bq_s, q_sb, 0, 512)])
        absorb([wv, bvrep, woA, woB])
        vt_group([0, 1])

        # epilogue state carried across blocks
        pending = []

        def emit_epilogue(o, sz, accA, accB):
            # acc layout per bank: [AV_e(32p); den_e(32p); AV_o(32p); den_o(32p)].
            # Full-bank aligned recip, then 32-part muls with the SBUF operand
            # cross-quadrant (PSUM+SBUF base mismatch is legal): att valid
            # lanes = AV*1/den; junk lanes stay memset-0.
            attA, attB = attA_sb, attB_sb
            for acc, rec, att in ((accA, recTA, attA), (accB, recTB, attB)):
                nc.vector.reciprocal_approx_fast(rec[:, :sz], acc[:, :sz])
                nc.vector.tensor_mul(att[0:32, :sz], acc[0:32, :sz], rec[32:64, :sz])
                nc.vector.tensor_mul(att[64:96, :sz], acc[64:96, :sz], rec[96:128, :sz])
            for cc in range(2):
                po = ps_a.tile([128, 512], f32, tag="acc", name="po")
                nc.tensor.matmul(
                    po[:, :sz], woA[:, 128 * cc : 128 * cc + 128], attA[:, :sz],
                    start=True, stop=False,
                )
                nc.tensor.matmul(
                    po[:, :sz], woB[:, 128 * cc : 128 * cc + 128], attB[:, :sz],
                    start=False, stop=True,
                )
                yt = trans.tile([128, 512], f32, tag="yt")
                nc.vector.tensor_copy(yt[:, :sz], po[:, :sz])
                nc.sync.dma_start(y_d[128 * cc : 128 * cc + 128, o : o + sz], yt[:, :sz])

        # block-0 interleave schedule: batched borrows of one score slot each
        interleave = {
            0: lambda: proj_group([(wk, xk_t, bk_s, k_sb, 512, 512),
                                   (wk, xk_t, bk_s, k_sb, 1024, 512)]),
            1: lambda: vt_group([2, 3, 4, 5]),
            2: lambda: proj_group([(wk, xk_t, bk_s, k_sb, 1536, 512),
                                   (wk, xk_t, bk_s, k_sb, 2048, 256)]),
            3: lambda: proj_group([(wq, xq_t, bq_s, q_sb, 512, 512),
                                   (wq, xq_t, bq_s, q_sb, 1024, 512)]),
            4: lambda: vt_group([6, 7, 8, 9]),
            5: lambda: proj_group([(wq, xq_t, bq_s, q_sb, 1536, 512),
                                   (wq, xq_t, bq_s, q_sb, 2048, 256)]),
            6: lambda: vt_group([10, 11, 12, 13]),
            7: lambda: vt_group([14, 15, 16, 17]),
        }

        # ---- attention ----
        for bi, (o, sz) in enumerate(NBLK):
            pend_av = []
            accbox = []

            def emit_av(item, accbox=accbox, sz=sz):
                # lazy acc allocation keeps the acc-tag rotation ordered
                # after the previous block's po tiles
                if not accbox:
                    accbox.append(ps_a.tile([128, 512], f32, tag="acc", name="accA"))
                    accbox.append(ps_a.tile([128, 512], f32, tag="acc", name="accB"))
                accA, accB = accbox
                j, ess = item
                # fused AV+den: stationary [vT_h | ones] (64 cols) produces
                # [AV_h (32p) ; den_h (32p)] per head in one matmul
                for hp in range(2):
                    es = ess[hp]
                    for hh in range(2):
                        h = 2 * hp + hh
                        acc = accA if h < 2 else accB
                        pa = 64 * (h % 2)
                        nc.tensor.matmul(
                            acc[pa : pa + 64, :sz],
                            vT2_sb[:, 256 * j + 64 * h : 256 * j + 64 * h + 64],
                            es[:, 512 * hh : 512 * hh + sz],
                            start=(j == 0), stop=(j == NMT - 1),
                            tile_position=(0, pa),
                        )

            for j in range(NMT):
                s2s = []
                for hp in range(2):  # head pair: heads (2*hp, 2*hp+1)
                    s2 = ps_s.tile([128, 1024], f32, tag="s")
                    s2s.append(s2)
                    for hh in range(2):
                        h = 2 * hp + hh
                        nc.tensor.matmul(
                            s2[:, 512 * hh : 512 * hh + sz],
                            k_sb[32 * h : 32 * h + 32, 128 * j : 128 * j + 128],
                            q_sb[32 * h : 32 * h + 32, o : o + sz],
                            start=True, stop=True, tile_position=(32 * h, 0),
                        )
                ess = []
                for hp in range(2):
                    s2 = s2s[hp]
                    es = espool.tile([128, 1024], bf16, tag="es")
                    ess.append(es)
                    # hp1 tiles go to DVE fast-exp on 2 of every 3 j's
                    on_dve = (hp == 1) and (j % 6 != 5)
                    if sz == 512:
                        if on_dve:
                            nc.vector.tensor_scalar(
                                es[:, :].bitcast(i16), s2[:, :], FE_A, FE_B, MULT, ADD)
                        else:
                            nc.scalar.activation(es[:, :], s2[:, :], EXP, scale=1.0 / 16.0)
                    else:
                        sv = s2[:, :].rearrange("p (b x) -> p b x", b=2)[:, :, :sz]
                        ev = es[:, :].rearrange("p (b x) -> p b x", b=2)[:, :, :sz]
                        if on_dve:
                            nc.vector.tensor_scalar(ev.bitcast(i16), sv, FE_A, FE_B, MULT, ADD)
                        else:
                            nc.scalar.activation(ev, sv, EXP, scale=1.0 / 16.0)
                pend_av.append((j, ess))
                if len(pend_av) > 2:
                    emit_av(pend_av.pop(0))
                # deferred epilogue of the previous block
                if j == 1 and pending:
                    emit_epilogue(*pending.pop())
                # block-0 interleave: finish projections + vT chunks
                if bi == 0 and j in interleave:
                    interleave[j]()
            while pend_av:
                emit_av(pend_av.pop(0))
            pending.append((o, sz, accbox[0], accbox[1]))
        emit_epilogue(*pending.pop())

    nc.compile()

    _PROG["nc"] = nc
    return nc


def make_in_maps(inputs):
    """Shard full inputs into the 8 per-core input maps."""
    import ml_dtypes

    bf16 = ml_dtypes.bfloat16
    g = {k: np.asarray(v) for k, v in inputs.items()}
    xq_b = [np.ascontiguousarray(g["queries"][b].reshape(CIN, N).astype(np.float16)) for b in range(B)]
    xk_b = [np.ascontiguousarray(g["keys"][b].reshape(CIN, N).astype(np.float16)) for b in range(B)]
    xv_b = [np.ascontiguousarray(g["values"][b].reshape(CIN, N).astype(np.float16)) for b in range(B)]
    def _wo_perm(woT, half):
        out = np.zeros((128, COUT), dtype=woT.dtype)
        out[0:32] = woT[64 * half : 64 * half + 32]
        out[64:96] = woT[64 * half + 32 : 64 * half + 64]
        return out
    in_maps = []
    for core in range(NCORES):
        b, grp = divmod(core, 2)
        hs = slice(grp * HGC, (grp + 1) * HGC)
        in_maps.append({
            "xq": xq_b[b],
            "xk": xk_b[b],
            "xv": xv_b[b],
            "wqT": np.ascontiguousarray(g["Wq"][hs, :].T.astype(np.float16)),
            "wkT": np.ascontiguousarray(g["Wk"][hs, :].T.astype(np.float16)),
            "wvT": np.ascontiguousarray(g["Wv"][hs, :].T.astype(np.float16)),
            "woA": _wo_perm(g["Wo"][:, hs].T.astype(bf16), 0),
            "woB": _wo_perm(g["Wo"][:, hs].T.astype(bf16), 1),
            "bq": np.ascontiguousarray(g["bq"][hs].reshape(HGC, 1).astype(np.float32)),
            "bk": np.ascontiguousarray(g["bk"][hs].reshape(HGC, 1).astype(np.float32)),
            "bvrep": np.ascontiguousarray(
                np.broadcast_to(g["bv"][hs].reshape(1, HGC), (128, HGC)).astype(np.float16)),
        })
    return in_maps


def unshard(results, bo):
    parts = [results[i]["y"] for i in range(NCORES)]
    out = np.empty((B, COUT, N), dtype=np.float32)
    for b in range(B):
        out[b] = parts[2 * b] + parts[2 * b + 1]
    out += np.asarray(bo, dtype=np.float32).reshape(1, COUT, 1)
    return out.reshape(B, COUT, HH, WW)


def kernel(**inputs):
    from concourse.bass_utils import run_bass_kernel_spmd

    nc = build_program()
    in_maps = make_in_maps(inputs)
    res = run_bass_kernel_spmd(nc, in_maps, list(range(NCORES)))
    return unshard(res.results, inputs["bo"])


# revision 14
# speedup vs baseline: 1.0887x; 1.0887x over previous
"""Trainium2 Bass kernel for MultiHeadGlobalAttention2d.

Sharding (8 cores): core = (batch b, head-group g), b in 0..3, g in 0..1.
Each core computes, for its batch and its 4 heads (128 channels):
  q/k/v projections, attention (softmax over keys), and the partial output
  projection  y_part = Wo[:, ch_slice] @ att_out.
Host sums the two partials per batch and adds the output bias.

v3 design (ACT-exp is the bottleneck engine; keep it 100% busy):
  - S^T orientation: score tiles [keys(m) on partitions, queries(n) on free];
    exp on ScalarE straight out of PSUM; denominators via ones-stationary
    matmuls accumulated alongside AV.
  - All projections in f16 (full PE rate + FWL); x inputs DMA'd as f16 in
    512-column chunks spread over many DMA queues so projections and the
    attention stream start within ~10us.
  - vT produced directly by PE matmul (x^T stationary, Wv^T moving) - no
    PE transposes. Biases added by DVE during the PSUM->SBUF copies.
  - Per-head accumulator open/close (start on j==0, stop on j==NMT-1).
  - PSUM: 3 score slots [128,1024] (6 banks) + 2 accumulator banks (AV+den).
    Leftover projection/vT work inside block 0 borrows score slots in
    batched groups (4 vT chunks or 2 proj blocks per borrow) to keep the
    QK lookahead intact.
  - Block o's epilogue (recip_approx + mul + out-proj) is emitted inside
    block o+1 so the exp stream never stalls at block boundaries.
"""

import numpy as np

B = 4
CIN = 256
COUT = 256
HH = 48
WW = 48
N = HH * WW            # 2304
D = 32                 # head dim
NHL = 4                # heads per core
HGC = NHL * D          # 128 channels per head-group
NCORES = 8
NBLK = [(0, 512), (512, 512), (1024, 512), (1536, 512), (2048, 256)]
NMT = N // 128         # 18 key tiles
XCH = [(0, 512), (512, 512), (1024, 512), (1536, 512), (2048, 256)]  # DMA chunks

_PROG = {}


def build_program():
    if "nc" in _PROG:
        return _PROG["nc"]

    from contextlib import ExitStack

    import concourse.bacc as bacc
    import concourse.mybir as mybir
    import concourse.tile as tile

    f32 = mybir.dt.float32
    bf16 = mybir.dt.bfloat16
    f16 = mybir.dt.float16
    i16 = mybir.dt.int16
    EXP = mybir.ActivationFunctionType.Exp
    MULT = mybir.AluOpType.mult
    ADD = mybir.AluOpType.add
    # DVE fast-exp: bitcast(int16(s*(128*log2e)/16 + (127*128 - C))) read as
    # bf16 approximates exp(s/16) (Schraudolph); C tuned for min max-rel-err.
    FE_A = 128.0 * 1.4426950408889634 / 16.0
    FE_B = 127.0 * 128.0 - 5.5

    nc = bacc.Bacc("TRN2", target_bir_lowering=False, debug=False)

    xq_d = nc.declare_dram_parameter("xq", [CIN, N], f16, False)
    xk_d = nc.declare_dram_parameter("xk", [CIN, N], f16, False)
    xv_d = nc.declare_dram_parameter("xv", [CIN, N], f16, False)
    wqT_d = nc.declare_dram_parameter("wqT", [CIN, HGC], f16, False)
    wkT_d = nc.declare_dram_parameter("wkT", [CIN, HGC], f16, False)
    wvT_d = nc.declare_dram_parameter("wvT", [CIN, HGC], f16, False)
    woA_d = nc.declare_dram_parameter("woA", [128, COUT], bf16, False)
    woB_d = nc.declare_dram_parameter("woB", [128, COUT], bf16, False)
    bq_d = nc.declare_dram_parameter("bq", [HGC, 1], f32, False)
    bk_d = nc.declare_dram_parameter("bk", [HGC, 1], f32, False)
    bvrep_d = nc.declare_dram_parameter("bvrep", [128, HGC], f16, False)
    y_d = nc.declare_dram_parameter("y", [COUT, N], f32, True)

    with tile.TileContext(nc) as tc, ExitStack() as ctx:
        const = ctx.enter_context(tc.tile_pool(name="const", bufs=1))
        resid = ctx.enter_context(tc.tile_pool(name="resid", bufs=1))
        xin = ctx.enter_context(tc.tile_pool(name="xin", bufs=1))
        espool = ctx.enter_context(tc.tile_pool(name="espool", bufs=12))
        trans = ctx.enter_context(tc.tile_pool(name="trans", bufs=2))
        ps_s = ctx.enter_context(tc.tile_pool(name="ps_s", bufs=3, space="PSUM"))
        ps_a = ctx.enter_context(tc.tile_pool(name="ps_a", bufs=2, space="PSUM"))

        # ---- constants / weights (small, issued first) ----
        wq = const.tile([128, 2, 128], f16)
        wk = const.tile([128, 2, 128], f16)
        wv = const.tile([128, 2, 128], f16)
        for wt, wd in ((wk, wkT_d), (wq, wqT_d), (wv, wvT_d)):
            nc.sync.dma_start(wt[:, :, :], wd[:, :].rearrange("(c p) k -> p c k", p=128))
        woA = const.tile([128, COUT], bf16)
        woB = const.tile([128, COUT], bf16)
        nc.sync.dma_start(woA[:, :], woA_d[:, :])
        nc.sync.dma_start(woB[:, :], woB_d[:, :])
        bq_s = const.tile([128, 1], f32)
        bk_s = const.tile([128, 1], f32)
        for bt, bd in ((bk_s, bk_d), (bq_s, bq_d)):
            nc.sync.dma_start(bt[:, :], bd[:, :])
        bvrep = const.tile([128, 128], f16)
        nc.sync.dma_start(bvrep[:, :], bvrep_d[:, :])

        # ---- x inputs: 512-col chunks, k/q/v interleaved so the front of
        # each tensor lands early and chunks spread across DMA queues ----
        xk_t = xin.tile([128, 2, N], f16)
        xq_t = xin.tile([128, 2, N], f16)
        xv_t = xin.tile([128, 2, N], f16)
        for lo, sz in XCH:
            for xt, xd in ((xk_t, xk_d), (xq_t, xq_d), (xv_t, xv_d)):
                nc.sync.dma_start(
                    xt[:, :, lo : lo + sz],
                    xd[:, lo : lo + sz].rearrange("(c p) n -> p c n", p=128),
                )

        # ---- residents ----
        q_sb = resid.tile([128, N], f16)
        k_sb = resid.tile([128, N], f16)
        vT2_sb = resid.tile([128, 2 * N], bf16)
        recTA = resid.tile([128, 512], f32)
        recTB = resid.tile([128, 512], f32)
        attA_sb = resid.tile([128, 512], bf16)
        attB_sb = resid.tile([128, 512], bf16)
        # ones strips (cols 32:64 of each 64-col head group), written once;
        # att junk lanes (den positions) stay zero forever
        nc.vector.memset(
            vT2_sb[:, :].rearrange("p (g t) -> p g t", t=64)[:, :, D:64], 1.0)
        nc.vector.memset(attA_sb[:, :], 0.0)
        nc.vector.memset(attB_sb[:, :], 0.0)

        # PE prologue absorb for const tiles consumed by PE (x chunks carry
        # their DMA wait on the consuming matmul directly).
        def absorb(tiles):
            scr = ps_s.tile([128, 512], f32, tag="s", name="scr")
            for t in tiles:
                tv = t[:, :, :].rearrange("p c k -> p (c k)") if t.ndim == 3 else t[:, :]
                F = tv.shape[-1]
                M = min(F, 128)
                W = min(F, 2)
                nc.tensor.matmul(
                    scr[0:M, 0:W], tv[0:1, F - M : F], tv[0:1, F - W : F],
                    start=True, stop=True,
                )

        def proj_piece(pp, po_, w, xt, brow, dst, o, sz):
            """dst[:, o:o+sz] = (W_h @ x)[:, o:o+sz] + b ; pp[:, po_:po_+sz] scratch."""
            nc.tensor.matmul(
                pp[:, po_ : po_ + sz], w[:, 0, :], xt[:, 0, o : o + sz],
                start=True, stop=False,
            )
            nc.tensor.matmul(
                pp[:, po_ : po_ + sz], w[:, 1, :], xt[:, 1, o : o + sz],
                start=False, stop=True,
            )
            nc.vector.tensor_scalar_add(dst[:, o : o + sz], pp[:, po_ : po_ + sz], brow[:, 0:1])

        def vt_piece(pt, po_, j):
            """vT_sb[:, 128j:+128] = (x_v[:, 128j:+128])^T @ Wv^T + bv."""
            nc.tensor.matmul(
                pt[:, po_ : po_ + 128], xv_t[:, 0, 128 * j : 128 * j + 128], wv[:, 0, :],
                start=True, stop=False,
            )
            nc.tensor.matmul(
                pt[:, po_ : po_ + 128], xv_t[:, 1, 128 * j : 128 * j + 128], wv[:, 1, :],
                start=False, stop=True,
            )
            vstrip = vT2_sb[:, 256 * j : 256 * j + 256].rearrange(
                "p (h t) -> p h t", h=NHL)[:, :, 0:D]
            nc.vector.tensor_add(
                vstrip,
                pt[:, po_ : po_ + 128].rearrange("p (h t) -> p h t", h=NHL),
                bvrep[:, :].rearrange("p (h t) -> p h t", h=NHL),
            )

        def vt_group(js):
            pt = ps_s.tile([128, 1024], f32, tag="s", name="pt")
            for i, j in enumerate(js):
                vt_piece(pt, 128 * i, j)

        def proj_group(pieces):
            pp = ps_s.tile([128, 1024], f32, tag="s", name="pp")
            po_ = 0
            for w, xt, brow, dst, o, sz in pieces:
                proj_piece(pp, po_, w, xt, brow, dst, o, sz)
                po_ += sz

        # ---- prologue: minimum to start the exp stream ----
        absorb([wk, wq, bk_s, bq_s])
        proj_group([(wk, xk_t, bk_s, k_sb, 0, 512)])
        proj_group([(wq, xq_t, bq_s, q_sb, 0, 512)])
        absorb([wv, bvrep, woA, woB])
        vt_group([0, 1])

        # epilogue state carried across blocks
        pending = []

        def emit_epilogue(o, sz, accA, accB):
            # acc layout per bank: [AV_e(32p); den_e(32p); AV_o(32p); den_o(32p)].
            # Full-bank aligned recip, then 32-part muls with the SBUF operand
            # cross-quadrant (PSUM+SBUF base mismatch is legal): att valid
            # lanes = AV*1/den; junk lanes stay memset-0.
            attA, attB = attA_sb, attB_sb
            for acc, rec, att in ((accA, recTA, attA), (accB, recTB, attB)):
                nc.vector.reciprocal_approx_fast(rec[:, :sz], acc[:, :sz])
                nc.vector.tensor_mul(att[0:32, :sz], acc[0:32, :sz], rec[32:64, :sz])
                nc.vector.tensor_mul(att[64:96, :sz], acc[64:96, :sz], rec[96:128, :sz])
            po = ps_s.tile([128, 1024], f32, tag="s", name="po")
            for cc in range(2):
                pv = po[:, 512 * cc : 512 * cc + sz]
                nc.tensor.matmul(
                    pv, woA[:, 128 * cc : 128 * cc + 128], attA[:, :sz],
                    start=True, stop=False,
                )
                nc.tensor.matmul(
                    pv, woB[:, 128 * cc : 128 * cc + 128], attB[:, :sz],
                    start=False, stop=True,
                )
                yt = trans.tile([128, 512], f32, tag="yt")
                nc.vector.tensor_copy(yt[:, :sz], pv)
                nc.sync.dma_start(y_d[128 * cc : 128 * cc + 128, o : o + sz], yt[:, :sz])

        # block-0 interleave schedule: batched borrows of one score slot each
        interleave = {
            0: lambda: proj_group([(wk, xk_t, bk_s, k_sb, 512, 512),
                                   (wk, xk_t, bk_s, k_sb, 1024, 512)]),
            1: lambda: vt_group([2, 3, 4, 5]),
            2: lambda: proj_group([(wk, xk_t, bk_s, k_sb, 1536, 512),
                                   (wk, xk_t, bk_s, k_sb, 2048, 256)]),
            3: lambda: proj_group([(wq, xq_t, bq_s, q_sb, 512, 512),
                                   (wq, xq_t, bq_s, q_sb, 1024, 512)]),
            4: lambda: vt_group([6, 7, 8, 9]),
            5: lambda: proj_group([(wq, xq_t, bq_s, q_sb, 1536, 512),
                                   (wq, xq_t, bq_s, q_sb, 2048, 256)]),
            6: lambda: vt_group([10, 11, 12, 13]),
            7: lambda: vt_group([14, 15, 16, 17]),
        }

        # ---- attention ----
        for bi, (o, sz) in enumerate(NBLK):
            accA = ps_a.tile([128, 512], f32, tag="acc", name="accA")
            accB = ps_a.tile([128, 512], f32, tag="acc", name="accB")
            pend_av = []

            def emit_av(item, accA=accA, accB=accB, sz=sz):
                j, ess = item
                # fused AV+den: stationary [vT_h | ones] (64 cols) produces
                # [AV_h (32p) ; den_h (32p)] per head in one matmul
                for hp in range(2):
                    es = ess[hp]
                    for hh in range(2):
                        h = 2 * hp + hh
                        acc = accA if h < 2 else accB
                        pa = 64 * (h % 2)
                        nc.tensor.matmul(
                            acc[pa : pa + 64, :sz],
                            vT2_sb[:, 256 * j + 64 * h : 256 * j + 64 * h + 64],
                            es[:, 512 * hh : 512 * hh + sz],
                            start=(j == 0), stop=(j == NMT - 1),
                            tile_position=(0, pa),
                        )

            for j in range(NMT):
                s2s = []
                for hp in range(2):  # head pair: heads (2*hp, 2*hp+1)
                    s2 = ps_s.tile([128, 1024], f32, tag="s")
                    s2s.append(s2)
                    for hh in range(2):
                        h = 2 * hp + hh
                        nc.tensor.matmul(
                            s2[:, 512 * hh : 512 * hh + sz],
                            k_sb[32 * h : 32 * h + 32, 128 * j : 128 * j + 128],
                            q_sb[32 * h : 32 * h + 32, o : o + sz],
                            start=True, stop=True, tile_position=(32 * h, 0),
                        )
                ess = []
                for hp in range(2):
                    s2 = s2s[hp]
                    es = espool.tile([128, 1024], bf16, tag="es")
                    ess.append(es)
                    # hp1 tiles go to DVE fast-exp on 2 of every 3 j's
                    on_dve = (hp == 1) and (j % 6 != 5)
                    if sz == 512:
                        if on_dve:
                            nc.vector.tensor_scalar(
                                es[:, :].bitcast(i16), s2[:, :], FE_A, FE_B, MULT, ADD)
                        else:
                            nc.scalar.activation(es[:, :], s2[:, :], EXP, scale=1.0 / 16.0)
                    else:
                        sv = s2[:, :].rearrange("p (b x) -> p b x", b=2)[:, :, :sz]
                        ev = es[:, :].rearrange("p (b x) -> p b x", b=2)[:, :, :sz]
                        if on_dve:
                            nc.vector.tensor_scalar(ev.bitcast(i16), sv, FE_A, FE_B, MULT, ADD)
                        else:
                            nc.scalar.activation(ev, sv, EXP, scale=1.0 / 16.0)
                pend_av.append((j, ess))
                if len(pend_av) > 2:
                    emit_av(pend_av.pop(0))
                # deferred epilogue of the previous block
                if j == 1 and pending:
                    emit_epilogue(*pending.pop())
                # block-0 interleave: finish projections + vT chunks
                if bi == 0 and j in interleave:
                    interleave[j]()
            while pend_av:
                emit_av(pend_av.pop(0))
            pending.append((o, sz, accA, accB))
        emit_epilogue(*pending.pop())

    nc.compile()

    _PROG["nc"] = nc
    return nc


def make_in_maps(inputs):
    """Shard full inputs into the 8 per-core input maps."""
    import ml_dtypes

    bf16 = ml_dtypes.bfloat16
    g = {k: np.asarray(v) for k, v in inputs.items()}
    xq_b = [np.ascontiguousarray(g["queries"][b].reshape(CIN, N).astype(np.float16)) for b in range(B)]
    xk_b = [np.ascontiguousarray(g["keys"][b].reshape(CIN, N).astype(np.float16)) for b in range(B)]
    xv_b = [np.ascontiguousarray(g["values"][b].reshape(CIN, N).astype(np.float16)) for b in range(B)]
    def _wo_perm(woT, half):
        out = np.zeros((128, COUT), dtype=woT.dtype)
        out[0:32] = woT[64 * half : 64 * half + 32]
        out[64:96] = woT[64 * half + 32 : 64 * half + 64]
        return out
    in_maps = []
    for core in range(NCORES):
        b, grp = divmod(core, 2)
        hs = slice(grp * HGC, (grp + 1) * HGC)
        in_maps.append({
            "xq": xq_b[b],
            "xk": xk_b[b],
            "xv": xv_b[b],
            "wqT": np.ascontiguousarray(g["Wq"][hs, :].T.astype(np.float16)),
            "wkT": np.ascontiguousarray(g["Wk"][hs, :].T.astype(np.float16)),
            "wvT": np.ascontiguousarray(g["Wv"][hs, :].T.astype(np.float16)),
            "woA": _wo_perm(g["Wo"][:, hs].T.astype(bf16), 0),
            "woB": _wo_perm(g["Wo"][:, hs].T.astype(bf16), 1),
            "bq": np.ascontiguousarray(g["bq"][hs].reshape(HGC, 1).astype(np.float32)),
            "bk": np.ascontiguousarray(g["bk"][hs].reshape(HGC, 1).astype(np.float32)),
            "bvrep": np.ascontiguousarray(
                np.broadcast_to(g["bv"][hs].reshape(1, HGC), (128, HGC)).astype(np.float16)),
        })
    return in_maps


def unshard(results, bo):
    parts = [results[i]["y"] for i in range(NCORES)]
    out = np.empty((B, COUT, N), dtype=np.float32)
    for b in range(B):
        out[b] = parts[2 * b] + parts[2 * b + 1]
    out += np.asarray(bo, dtype=np.float32).reshape(1, COUT, 1)
    return out.reshape(B, COUT, HH, WW)


def kernel(**inputs):
    from concourse.bass_utils import run_bass_kernel_spmd

    nc = build_program()
    in_maps = make_in_maps(inputs)
    res = run_bass_kernel_spmd(nc, in_maps, list(range(NCORES)))
    return unshard(res.results, inputs["bo"])
